# revision 30
# baseline (speedup 1.0000x reference)
"""Trainium2 Bass kernel for nn_AttnGlobal (B=8, N=4096, DIM=128).

reference:
    kv = x @ Wkv + bkv ; k, v = split(kv)
    q = q_global / sqrt(d)
    scores = einsum("bnd,bmd->bnm", k, q)       # softmax over m
    attn = softmax(scores, axis=-1)
    out = einsum("bnm,bmd->bnd", attn, v) @ Wp + bp

Sharding: pure data-parallel over B across the 8 cores (one batch each).

Host-side algebra folds:
    w   = x @ (Wv @ Wp)            (since attn @ (x@Wv) @ Wp = attn @ (x@(Wv@Wp)))
    bpe = bv @ Wp + bp             (since rows of attn sum to 1)

Per-core dataflow:
    xT, qT  : host-pretransposed fp16 inputs        [d, n] / [d, m]
    kT      = Wk.T @ xT + bk                        [d, n]   fp16
    S.T     = qT.T-tiles @ kT                       [m, n] tiles in PSUM (fp32)
    E.T     = exp(S.T / sqrt(d))                    fp16, ACT straight from PSUM
    U_aug   = E @ [w | 1]                           [n, 129] accumulated in PSUM
    out     = U[:, :128] * (1 / U[:, 128]) + bpe    DVE, then DMA out

Schedule: the ACT engine's exp stream (16.8M elems/core @ 1 elem/cyc/lane)
is the hard floor (~128us). Everything else is arranged to keep ACT
gapless:

- S-group matmuls are slot-paced: S(c,i) can only run once exp(c,i-2) has
  freed its PSUM slot (two pools in strict alternation, even group count
  per chunk so the alternation is seamless across chunk boundaries).
  U-work is emitted as small 2-tile units threaded between S-groups by an
  ACT-time-proportional owed-work scheduler, deferred past the scratch
  phase so the shared "ps" pool ring (scratch first, then long-held U
  accumulators) can never deadlock the in-order PE queue.
- Input DMAs are cut into pieces sized to land just before their
  need-time, spread over the sync/gpsimd/scalar queues (~30-45GB/s each):
  qT alone outruns one queue during window 0 so its pieces alternate
  between gpsimd and scalar; the kT bias rides inside the wkb transfer
  (a separate 128x4B-descriptor DMA took ~3us and stalled the first exp).
- kT and w-tile scratch matmuls execute inside window 0-1's DMA-bound
  slack; per-chunk outputs leave via one 3D DMA from an SBUF staging
  buffer.
"""

import os
import sys

try:
    import concourse  # noqa: F401  (resolvable via PYTHONPATH on axon images)
except ImportError:
    for _p in ("/opt/trn_rl_repo", os.path.expanduser("~/.axon_site/_ro/trn_rl_repo")):
        if os.path.isdir(_p) and _p not in sys.path:
            sys.path.append(_p)

import numpy as np

import concourse.bacc as bacc
import concourse.mybir as mybir
from concourse.bass_utils import run_bass_kernel_spmd
from concourse.tile import TileContext

B, N, D = 8, 4096, 128
NT = N // 128          # 32 row tiles
NC = N // 512          # 8 column chunks
F32 = mybir.dt.float32
F16 = mybir.dt.float16
EXP_SCALE = 1.0 / float(np.sqrt(D))

# 12 score-groups per chunk; even index -> st2 pool (max 2 tiles), odd ->
# st4 (max 4 tiles). Small groups lead each chunk so chunk 0's early exps
# only need the first qT DMA pieces; the even count keeps the st2/st4
# alternation seamless across chunk boundaries.
GROUPS = [2, 2, 2, 2, 2, 4, 2, 4, 2, 4, 2, 4]
assert sum(GROUPS) == NT
NG = len(GROUPS)
GSTART = [sum(GROUPS[:i]) for i in range(NG)]
# ET/w_aug slot -> logical m-tile: groups consume qT pieces in arrival
# order (gpsimd: p0,p1,p2,p4 / scalar: p3,p5,p7,p6), so chunk 0's exp
# stream never waits on a late piece
TPERM = ([0, 1, 2, 3] + [4, 5, 6, 7] + [12, 13, 14, 15] + [8, 9, 10, 11]
         + [20, 21, 22, 23] + [16, 17, 18, 19] + [28, 29, 30, 31]
         + [24, 25, 26, 27])
# w-quad q (w_aug slots 4q..4q+3) reads this xT chunk
WQSRC = [t // 4 for t in TPERM[::4]]


def _chunk_groups(c):
    return list(zip(GSTART, GROUPS))


def build(reps: int = 1):
    """Build and compile the per-core Bass program (identical on all cores)."""
    nc = bacc.Bacc("TRN2", target_bir_lowering=False)

    xt = nc.dram_tensor("xt", [D, N], F16, kind="ExternalInput")
    qt = nc.dram_tensor("qt", [D, N], F16, kind="ExternalInput")
    wkb = nc.dram_tensor("wkb", [D, 130], F16, kind="ExternalInput")
    wvp = nc.dram_tensor("wvp", [D, D], F16, kind="ExternalInput")
    bpe = nc.dram_tensor("bpe", [D, D], F32, kind="ExternalInput")  # row-tiled bias
    out = nc.dram_tensor("out", [N, D], F32, kind="ExternalOutput")

    with TileContext(nc) as tc:
        xTc = [nc.alloc_sbuf_tensor(f"xT{c}", [128, 512], F16) for c in range(NC)]
        qTp = [nc.alloc_sbuf_tensor(f"qT{p}", [128, 512], F16) for p in range(NC)]
        kTc = [nc.alloc_sbuf_tensor(f"kT{c}", [128, 512], F16) for c in range(NC)]
        w_aug = nc.alloc_sbuf_tensor("w_aug", [128, NT, 130], F16)
        ET = [nc.alloc_sbuf_tensor(f"et{i}", [128, NT, 512], F16) for i in range(3)]
        warm_sb = nc.alloc_sbuf_tensor("warm_sb", [128, 512], F16)
        wkb_sb = nc.alloc_sbuf_tensor("wkb_sb", [128, 130], F16)
        wvp_sb = nc.alloc_sbuf_tensor("wvp_sb", [128, 128], F16)
        stage = [nc.alloc_sbuf_tensor(f"stg{i}", [128, 4, 128], F32)
                 for i in range(2)]
        bk32 = nc.alloc_sbuf_tensor("bk32", [128, 1], F32)
        bpe_sb = nc.alloc_sbuf_tensor("bpe_sb", [128, 128], F32)

        # DMA piece plan (each queue streams sequentially at ~45GB/s;
        # ~2.8us per 128KB piece): qT pieces alternate between gpsimd and
        # scalar so the m-tile arrival rate keeps up with the exp stream;
        # xT rides sync (chunk 0 first for kT(0)), with the last two
        # chunks on gpsimd after its qT pieces are out.
        def xp(c):
            return (xt[:, c * 512:(c + 1) * 512], xTc[c][:])

        def qp(p):
            return (qt[:, p * 512:(p + 1) * 512], qTp[p][:])

        nc.sync.dma_start(xTc[0][:, :256], xt[:, 0:256])
        nc.sync.dma_start(xTc[0][:, 256:], xt[:, 256:512])
        for s_, d_ in [xp(1), xp(2), xp(3), xp(4), xp(5)]:
            nc.sync.dma_start(d_, s_)
        nc.gpsimd.dma_start(qTp[0][:, :256], qt[:, 0:256])
        nc.gpsimd.dma_start(qTp[0][:, 256:], qt[:, 256:512])
        for s_, d_ in [qp(1)]:
            nc.gpsimd.dma_start(d_, s_)
        nc.gpsimd.dma_start(wvp_sb[:], wvp[:])
        for s_, d_ in [qp(2), qp(4), xp(6), xp(7)]:
            nc.gpsimd.dma_start(d_, s_)
        nc.scalar.dma_start(wkb_sb[:], wkb[:])
        for s_, d_ in [qp(3), qp(5), qp(7), qp(6)]:
            nc.scalar.dma_start(d_, s_)
        nc.scalar.dma_start(bpe_sb[:], bpe[:])

        with (
            tc.tile_pool(name="outp", bufs=4) as outp,
            tc.tile_pool(name="small", bufs=4) as small,
            tc.tile_pool(name="ps", bufs=2, space="PSUM") as psh,
            tc.tile_pool(name="st4", bufs=1, space="PSUM") as st4,
            tc.tile_pool(name="st2", bufs=1, space="PSUM") as st2,
        ):
            uacc = {}

            def s_group(c, mt, g, gi):
                """scores S.T [m-tiles mt..mt+g, n-chunk c] -> exp -> E.T"""
                pool, tag = (st2, "st2") if gi % 2 == 0 else (st4, "st4")
                stp = pool.tile([128, g * 512], F32, tag=tag)
                for i in range(g):
                    m = TPERM[mt + i]
                    nc.tensor.matmul(
                        stp[:, i * 512:(i + 1) * 512],
                        qTp[m // 4][:, (m % 4) * 128:(m % 4 + 1) * 128],
                        kTc[c][:],
                    )
                nc.scalar.activation(
                    ET[c % 3][:, mt:mt + g, :],
                    stp[:],
                    mybir.ActivationFunctionType.Exp,
                    scale=EXP_SCALE,
                )

            def u_unit(c, j):
                """U += E.T-tiles[2j..2j+1].T @ [w | 1] for output chunk c."""
                if c not in uacc:
                    upa = psh.tile([128, 512], F32, tag="ps")
                    upb = psh.tile([128, 512], F32, tag="ps")
                    uacc[c] = (upa, upb)
                ups = uacc[c]
                buf = ET[c % 3]
                for t in (2 * j, 2 * j + 1):
                    for jj in range(4):
                        up = ups[jj // 2]
                        off = 129 * (jj % 2)
                        nc.tensor.matmul(
                            up[:, off:off + 129],
                            buf[:, t, jj * 128:(jj + 1) * 128],
                            w_aug[:, t, :129],
                            start=(t == 0 and jj % 2 == 0),
                            stop=(t == NT - 1 and jj % 2 == 1),
                        )

            def u_final(c):
                """normalize U by its ones-column, add bias, DMA out."""
                ups = uacc.pop(c)
                stg = stage[c % 2]
                last = c == NC - 1
                for j in range(4):
                    up = ups[j // 2]
                    off = 129 * (j % 2)
                    rec = small.tile([128, 1], F32, tag="rec")
                    nc.vector.reciprocal(rec[:], up[:, off + 128:off + 129])
                    nc.vector.scalar_tensor_tensor(
                        stg[:, j, :],
                        up[:, off:off + 128],
                        rec[:],
                        bpe_sb[:],
                        mybir.AluOpType.mult,
                        mybir.AluOpType.add,
                    )
                    if last:
                        # tail chunk: ship each row-tile as soon as its
                        # normalize lands instead of waiting for all four
                        row = c * 512 + j * 128
                        nc.sync.dma_start(out[row:row + 128, :], stg[:, j, :])
                if not last:
                    nc.sync.dma_start(
                        out[c * 512:(c + 1) * 512, :].rearrange(
                            "(j p) d -> p j d", p=128
                        ),
                        stg[:],
                    )

            def kT_ps(c):
                kt = psh.tile([128, 512], F32, tag="ps")
                nc.tensor.matmul(kt[:], wkb_sb[:, :128], xTc[c][:])
                nc.vector.tensor_scalar_add(kTc[c][:], kt[:], bk32[:])

            def w_quad(q):
                # w_aug slots 4q..4q+3 (logical tiles TPERM[...], all from
                # one xT chunk); one strided DVE copy into w_aug
                wp = psh.tile([128, 512], F32, tag="ps")
                for i in range(4):
                    m = TPERM[4 * q + i]
                    nc.tensor.matmul(
                        wp[:, i * 128:(i + 1) * 128],
                        xTc[WQSRC[q]][:, (m % 4) * 128:(m % 4 + 1) * 128],
                        wvp_sb[:],
                    )
                nc.vector.tensor_copy(w_aug[:, 4 * q:4 * q + 4, :128], wp[:])

            def body(_iv=None):
                # HAM warmup: data-independent matmuls keep the PE busy
                # while the first input DMAs land, so the clock gate
                # un-throttles (K=4/8 -> 8/8) before the real work starts.
                nc.vector.memset(warm_sb[:], 0.0)
                warm = psh.tile([128, 512], F32, tag="ps")
                for _ in range(6):
                    nc.tensor.matmul(warm[:], warm_sb[:, :128], warm_sb[:])
                nc.vector.memset(w_aug[:, :, 128:129], 1.0)
                nc.vector.tensor_copy(bk32[:], wkb_sb[:, 128:129])
                kt0 = psh.tile([128, 512], F32, tag="ps")
                nc.tensor.matmul(kt0[:, :256], wkb_sb[:, :128], xTc[0][:, :256])
                nc.tensor.matmul(kt0[:, 256:], wkb_sb[:, :128], xTc[0][:, 256:])
                nc.vector.tensor_scalar_add(kTc[0][:], kt0[:], bk32[:])

                # kT/w-quad scratch is threaded through the early slot
                # stream (window 0-1 exp pace is DMA-arrival-bound, so the
                # scratch matmuls execute inside its slack), each placed at
                # a slot by which its xT piece has landed. U-units are
                # deferred until after the last scratch emission, which
                # guarantees every "ps"-ring scratch allocation precedes
                # the first long-held U accumulator: the in-order PE queue
                # can never deadlock on the pool ring.
                wq_emitted = [False] * 8

                def WQ(q):
                    def f():
                        w_quad(q)
                        wq_emitted[q] = True
                    return f

                scratch = {
                    1: [WQ(0)],
                    2: [lambda: kT_ps(1)],
                    3: [WQ(1)],
                    4: [lambda: kT_ps(2)],
                    5: [WQ(3)],
                    6: [lambda: kT_ps(3)],
                    7: [WQ(2)],
                    12: [WQ(5)],
                    13: [WQ(4)],
                    14: [WQ(7), lambda: kT_ps(4)],
                    15: [WQ(6), lambda: kT_ps(6)],
                    16: [lambda: kT_ps(5)],
                    17: [lambda: kT_ps(7)],
                }

                seq = [(c, gi) for c in range(NC) for gi in range(NG)]
                tile_group = {}   # (c, mt) -> global slot index of its group
                for k, (c, gi) in enumerate(seq):
                    mt, g = _chunk_groups(c)[gi]
                    for t in range(mt, mt + g):
                        tile_group[(c, t)] = k

                # ACT-time-proportional unit pacing, starting after the
                # last scratch slot (17): weight per slot ~ FD + overhead
                wts = [GROUPS[gi] * 512 + 300 for (c, gi) in seq]
                cum = [0]
                for w_ in wts:
                    cum.append(cum[-1] + w_)
                U0, U1 = cum[18], cum[len(seq)]

                units = [(c, j) for c in range(NC) for j in range(NT // 2)]
                emitted = 0

                def unit_ready(k):
                    if emitted >= len(units):
                        return False
                    c, j = units[emitted]
                    if tile_group[(c, 2 * j + 1)] > k - 2:
                        return False
                    return c > 0 or wq_emitted[(2 * j + 1) // 4]

                def emit_unit():
                    nonlocal emitted
                    uc, uj = units[emitted]
                    u_unit(uc, uj)
                    emitted += 1
                    if uj == NT // 2 - 1:
                        u_final(uc)

                for k, (c, gi) in enumerate(seq):
                    mt, g = _chunk_groups(c)[gi]
                    s_group(c, mt, g, gi)
                    for fn in scratch.get(k, ()):
                        fn()
                    target = min(
                        len(units),
                        max(0, (len(units) * (cum[k + 1] - U0)) // (U1 - U0)),
                    )
                    if gi == 4:
                        # next slot is the chunk's first 4-tile S-group; its
                        # PSUM slot frees two small exps back with no PE
                        # lookahead surplus -- keep the queue clear so the
                        # S-matmuls land before the ACT drains exp(c,4)
                        continue
                    while emitted < target and unit_ready(k):
                        emit_unit()
                while emitted < len(units):
                    emit_unit()

            if reps == 1:
                body()
            else:
                with tc.For_i(0, reps, 1):
                    body()

    nc.compile()
    return nc


def _prep_weights(Wkv, bkv, Wp, bp):
    Wkv = np.asarray(Wkv, np.float32)
    bkv = np.asarray(bkv, np.float32)
    Wp = np.asarray(Wp, np.float32)
    bp = np.asarray(bp, np.float32)
    wkb = np.zeros((D, 130), np.float16)
    wkb[:, :D] = Wkv[:, :D].astype(np.float16)
    wkb[:, D] = bkv[:D].astype(np.float16)
    wvp = np.ascontiguousarray((Wkv[:, D:] @ Wp).astype(np.float16))
    bpe_row = bkv[D:] @ Wp + bp
    bpe = np.ascontiguousarray(np.tile(bpe_row[None, :], (D, 1)))
    return wkb, wvp, bpe


_NC_CACHE = {}


def kernel(x, q_global, Wkv, bkv, Wp, bp):
    xt = np.asarray(x, np.float32).astype(np.float16).transpose(0, 2, 1)
    qt = np.asarray(q_global, np.float32).astype(np.float16).transpose(0, 2, 1)
    wkb, wvp, bpe = _prep_weights(Wkv, bkv, Wp, bp)

    if 1 not in _NC_CACHE:
        _NC_CACHE[1] = build(reps=1)
    nc = _NC_CACHE[1]

    in_maps = [
        {
            "xt": np.ascontiguousarray(xt[b]),
            "qt": np.ascontiguousarray(qt[b]),
            "wkb": wkb,
            "wvp": wvp,
            "bpe": bpe,
        }
        for b in range(B)
    ]
    res = run_bass_kernel_spmd(nc, in_maps, core_ids=list(range(B)))
    return np.stack([res.results[b]["out"] for b in range(B)], axis=0)


# revision 32
# speedup vs baseline: 1.0036x; 1.0036x over previous
"""Trainium2 Bass kernel for nn_AttnGlobal (B=8, N=4096, DIM=128).

reference:
    kv = x @ Wkv + bkv ; k, v = split(kv)
    q = q_global / sqrt(d)
    scores = einsum("bnd,bmd->bnm", k, q)       # softmax over m
    attn = softmax(scores, axis=-1)
    out = einsum("bnm,bmd->bnd", attn, v) @ Wp + bp

Sharding: pure data-parallel over B across the 8 cores (one batch each).

Host-side algebra folds:
    w   = x @ (Wv @ Wp)            (since attn @ (x@Wv) @ Wp = attn @ (x@(Wv@Wp)))
    bpe = bv @ Wp + bp             (since rows of attn sum to 1)

Per-core dataflow:
    xT, qT  : host-pretransposed fp16 inputs        [d, n] / [d, m]
    kT      = Wk.T @ xT + bk                        [d, n]   fp16
    S.T     = qT.T-tiles @ kT                       [m, n] tiles in PSUM (fp32)
    E.T     = exp(S.T / sqrt(d))                    fp16, ACT straight from PSUM
    U_aug   = E @ [w | 1]                           [n, 129] accumulated in PSUM
    out     = U[:, :128] * (1 / U[:, 128]) + bpe    DVE, then DMA out

Schedule: the ACT engine's exp stream (16.8M elems/core @ 1 elem/cyc/lane)
is the hard floor (~128us). Everything else is arranged to keep ACT
gapless:

- S-group matmuls are slot-paced: S(c,i) can only run once exp(c,i-2) has
  freed its PSUM slot (two pools in strict alternation, even group count
  per chunk so the alternation is seamless across chunk boundaries).
  U-work is emitted as small 2-tile units threaded between S-groups by an
  ACT-time-proportional owed-work scheduler, deferred past the scratch
  phase so the shared "ps" pool ring (scratch first, then long-held U
  accumulators) can never deadlock the in-order PE queue.
- Input DMAs are cut into pieces sized to land just before their
  need-time, spread over the sync/gpsimd/scalar queues (~30-45GB/s each):
  qT alone outruns one queue during window 0 so its pieces alternate
  between gpsimd and scalar; the kT bias rides inside the wkb transfer
  (a separate 128x4B-descriptor DMA took ~3us and stalled the first exp).
- kT and w-tile scratch matmuls execute inside window 0-1's DMA-bound
  slack; per-chunk outputs leave via one 3D DMA from an SBUF staging
  buffer.
"""

import os
import sys

try:
    import concourse  # noqa: F401  (resolvable via PYTHONPATH on axon images)
except ImportError:
    for _p in ("/opt/trn_rl_repo", os.path.expanduser("~/.axon_site/_ro/trn_rl_repo")):
        if os.path.isdir(_p) and _p not in sys.path:
            sys.path.append(_p)

import numpy as np

import concourse.bacc as bacc
import concourse.mybir as mybir
from concourse.bass_utils import run_bass_kernel_spmd
from concourse.tile import TileContext

B, N, D = 8, 4096, 128
NT = N // 128          # 32 row tiles
NC = N // 512          # 8 column chunks
F32 = mybir.dt.float32
F16 = mybir.dt.float16
EXP_SCALE = 1.0 / float(np.sqrt(D))

# 12 score-groups per chunk; even index -> st2 pool (max 2 tiles), odd ->
# st4 (max 4 tiles). Small groups lead each chunk so chunk 0's early exps
# only need the first qT DMA pieces; the even count keeps the st2/st4
# alternation seamless across chunk boundaries.
GROUPS = [1, 2, 2, 4, 2, 4, 2, 4, 2, 4, 1, 4]
assert sum(GROUPS) == NT
NG = len(GROUPS)
GSTART = [sum(GROUPS[:i]) for i in range(NG)]


def _chunk_groups(c):
    return list(zip(GSTART, GROUPS))


def build(reps: int = 1):
    """Build and compile the per-core Bass program (identical on all cores)."""
    nc = bacc.Bacc("TRN2", target_bir_lowering=False)

    xt = nc.dram_tensor("xt", [D, N], F16, kind="ExternalInput")
    qt = nc.dram_tensor("qt", [D, N], F16, kind="ExternalInput")
    wkb = nc.dram_tensor("wkb", [D, 130], F16, kind="ExternalInput")
    wvp = nc.dram_tensor("wvp", [D, D], F16, kind="ExternalInput")
    bpe = nc.dram_tensor("bpe", [D, D], F32, kind="ExternalInput")  # row-tiled bias
    out = nc.dram_tensor("out", [N, D], F32, kind="ExternalOutput")

    with TileContext(nc) as tc:
        xTc = [nc.alloc_sbuf_tensor(f"xT{c}", [128, 512], F16) for c in range(NC)]
        qTp = [nc.alloc_sbuf_tensor(f"qT{p}", [128, 512], F16) for p in range(NC)]
        kTc = [nc.alloc_sbuf_tensor(f"kT{c}", [128, 512], F16) for c in range(NC)]
        w_aug = nc.alloc_sbuf_tensor("w_aug", [128, NT, 130], F16)
        ET = [nc.alloc_sbuf_tensor(f"et{i}", [128, NT, 512], F16) for i in range(3)]
        warm_sb = nc.alloc_sbuf_tensor("warm_sb", [128, 512], F16)
        wkb_sb = nc.alloc_sbuf_tensor("wkb_sb", [128, 130], F16)
        wvp_sb = nc.alloc_sbuf_tensor("wvp_sb", [128, 128], F16)
        stage = [nc.alloc_sbuf_tensor(f"stg{i}", [128, 4, 128], F32)
                 for i in range(2)]
        bk32 = nc.alloc_sbuf_tensor("bk32", [128, 1], F32)
        bpe_sb = nc.alloc_sbuf_tensor("bpe_sb", [128, 128], F32)

        # DMA piece plan (each queue streams sequentially at ~45GB/s;
        # ~2.8us per 128KB piece): qT pieces alternate between gpsimd and
        # scalar so the m-tile arrival rate keeps up with the exp stream;
        # xT rides sync (chunk 0 first for kT(0)), with the last two
        # chunks on gpsimd after its qT pieces are out.
        def xp(c):
            return (xt[:, c * 512:(c + 1) * 512], xTc[c][:])

        def qp(p):
            return (qt[:, p * 512:(p + 1) * 512], qTp[p][:])

        nc.sync.dma_start(xTc[0][:, :256], xt[:, 0:256])
        nc.sync.dma_start(xTc[0][:, 256:], xt[:, 256:512])
        for s_, d_ in [xp(1), xp(2), xp(3), xp(4), xp(5)]:
            nc.sync.dma_start(d_, s_)
        nc.gpsimd.dma_start(qTp[0][:, :256], qt[:, 0:256])
        nc.gpsimd.dma_start(qTp[0][:, 256:], qt[:, 256:512])
        nc.gpsimd.dma_start(wvp_sb[:], wvp[:])
        for s_, d_ in [qp(2), qp(4), qp(6), xp(6), xp(7)]:
            nc.gpsimd.dma_start(d_, s_)
        nc.scalar.dma_start(wkb_sb[:], wkb[:])
        for s_, d_ in [qp(1), qp(3), qp(5), qp(7)]:
            nc.scalar.dma_start(d_, s_)
        nc.scalar.dma_start(bpe_sb[:], bpe[:])

        with (
            tc.tile_pool(name="outp", bufs=4) as outp,
            tc.tile_pool(name="small", bufs=4) as small,
            tc.tile_pool(name="ps", bufs=2, space="PSUM") as psh,
            tc.tile_pool(name="st4", bufs=1, space="PSUM") as st4,
            tc.tile_pool(name="st2", bufs=1, space="PSUM") as st2,
        ):
            uacc = {}

            def s_group(c, mt, g, gi):
                """scores S.T [m-tiles mt..mt+g, n-chunk c] -> exp -> E.T"""
                pool, tag = (st2, "st2") if gi % 2 == 0 else (st4, "st4")
                stp = pool.tile([128, g * 512], F32, tag=tag)
                for i in range(g):
                    m = mt + i
                    nc.tensor.matmul(
                        stp[:, i * 512:(i + 1) * 512],
                        qTp[m // 4][:, (m % 4) * 128:(m % 4 + 1) * 128],
                        kTc[c][:],
                    )
                nc.scalar.activation(
                    ET[c % 3][:, mt:mt + g, :],
                    stp[:],
                    mybir.ActivationFunctionType.Exp,
                    scale=EXP_SCALE,
                )

            def u_unit(c, j):
                """U += E.T-tiles[2j..2j+1].T @ [w | 1] for output chunk c."""
                if c not in uacc:
                    upa = psh.tile([128, 512], F32, tag="ps")
                    upb = psh.tile([128, 512], F32, tag="ps")
                    uacc[c] = (upa, upb)
                ups = uacc[c]
                buf = ET[c % 3]
                for t in (2 * j, 2 * j + 1):
                    for jj in range(4):
                        up = ups[jj // 2]
                        off = 129 * (jj % 2)
                        nc.tensor.matmul(
                            up[:, off:off + 129],
                            buf[:, t, jj * 128:(jj + 1) * 128],
                            w_aug[:, t, :129],
                            start=(t == 0 and jj % 2 == 0),
                            stop=(t == NT - 1 and jj % 2 == 1),
                        )

            def u_final(c):
                """normalize U by its ones-column, add bias, DMA out."""
                ups = uacc.pop(c)
                stg = stage[c % 2]
                last = c == NC - 1
                for j in range(4):
                    up = ups[j // 2]
                    off = 129 * (j % 2)
                    rec = small.tile([128, 1], F32, tag="rec")
                    nc.vector.reciprocal(rec[:], up[:, off + 128:off + 129])
                    nc.vector.scalar_tensor_tensor(
                        stg[:, j, :],
                        up[:, off:off + 128],
                        rec[:],
                        bpe_sb[:],
                        mybir.AluOpType.mult,
                        mybir.AluOpType.add,
                    )
                    if last:
                        # tail chunk: ship each row-tile as soon as its
                        # normalize lands instead of waiting for all four
                        row = c * 512 + j * 128
                        nc.sync.dma_start(out[row:row + 128, :], stg[:, j, :])
                if not last:
                    nc.sync.dma_start(
                        out[c * 512:(c + 1) * 512, :].rearrange(
                            "(j p) d -> p j d", p=128
                        ),
                        stg[:],
                    )

            def kT_ps(c):
                kt = psh.tile([128, 512], F32, tag="ps")
                nc.tensor.matmul(kt[:], wkb_sb[:, :128], xTc[c][:])
                nc.vector.tensor_scalar_add(kTc[c][:], kt[:], bk32[:])

            def w_quad(q):
                # w-tiles 4q..4q+3; one strided DVE copy into w_aug
                wp = psh.tile([128, 512], F32, tag="ps")
                for i in range(4):
                    t = 4 * q + i
                    nc.tensor.matmul(
                        wp[:, i * 128:(i + 1) * 128],
                        xTc[q][:, i * 128:(i + 1) * 128],
                        wvp_sb[:],
                    )
                nc.vector.tensor_copy(w_aug[:, 4 * q:4 * q + 4, :128], wp[:])

            def body(_iv=None):
                # HAM warmup: data-independent matmuls keep the PE busy
                # while the first input DMAs land, so the clock gate
                # un-throttles (K=4/8 -> 8/8) before the real work starts.
                nc.vector.memset(warm_sb[:], 0.0)
                warm = psh.tile([128, 512], F32, tag="ps")
                for _ in range(6):
                    nc.tensor.matmul(warm[:], warm_sb[:, :128], warm_sb[:])
                for _ in range(8):
                    nc.tensor.matmul(
                        warm[:, :128], warm_sb[:, :128], warm_sb[:, :128]
                    )
                nc.vector.memset(w_aug[:, :, 128:129], 1.0)
                nc.vector.tensor_copy(bk32[:], wkb_sb[:, 128:129])
                kt0 = psh.tile([128, 512], F32, tag="ps")
                nc.tensor.matmul(kt0[:, :256], wkb_sb[:, :128], xTc[0][:, :256])
                nc.tensor.matmul(kt0[:, 256:], wkb_sb[:, :128], xTc[0][:, 256:])
                nc.vector.tensor_scalar_add(kTc[0][:], kt0[:], bk32[:])

                # kT/w-quad scratch is threaded through the early slot
                # stream (window 0-1 exp pace is DMA-arrival-bound, so the
                # scratch matmuls execute inside its slack), each placed at
                # a slot by which its xT piece has landed. U-units are
                # deferred until after the last scratch emission, which
                # guarantees every "ps"-ring scratch allocation precedes
                # the first long-held U accumulator: the in-order PE queue
                # can never deadlock on the pool ring.
                wq_done = [0]

                def WQ(q):
                    def f():
                        w_quad(q)
                        wq_done[0] = q + 1
                    return f

                scratch = {
                    1: [WQ(0)],
                    2: [lambda: kT_ps(1)],
                    3: [WQ(1)],
                    4: [lambda: kT_ps(2)],
                    5: [WQ(2)],
                    6: [lambda: kT_ps(3)],
                    7: [WQ(3)],
                    12: [WQ(4)],
                    13: [WQ(5)],
                    14: [WQ(6), lambda: kT_ps(4)],
                    15: [WQ(7), lambda: kT_ps(6)],
                    16: [lambda: kT_ps(5)],
                    17: [lambda: kT_ps(7)],
                }

                seq = [(c, gi) for c in range(NC) for gi in range(NG)]
                tile_group = {}   # (c, mt) -> global slot index of its group
                for k, (c, gi) in enumerate(seq):
                    mt, g = _chunk_groups(c)[gi]
                    for t in range(mt, mt + g):
                        tile_group[(c, t)] = k

                # ACT-time-proportional unit pacing, starting after the
                # last scratch slot (17): weight per slot ~ FD + overhead
                wts = [GROUPS[gi] * 512 + 300 for (c, gi) in seq]
                cum = [0]
                for w_ in wts:
                    cum.append(cum[-1] + w_)
                U0, U1 = cum[18], cum[len(seq)]

                units = [(c, j) for c in range(NC) for j in range(NT // 2)]
                emitted = 0

                def unit_ready(k):
                    if emitted >= len(units):
                        return False
                    c, j = units[emitted]
                    if tile_group[(c, 2 * j + 1)] > k - 2:
                        return False
                    return (2 * j + 1) // 4 < wq_done[0] or c > 0

                def emit_unit():
                    nonlocal emitted
                    uc, uj = units[emitted]
                    u_unit(uc, uj)
                    emitted += 1
                    if uj == NT // 2 - 1:
                        u_final(uc)

                for k, (c, gi) in enumerate(seq):
                    mt, g = _chunk_groups(c)[gi]
                    s_group(c, mt, g, gi)
                    for fn in scratch.get(k, ()):
                        fn()
                    target = min(
                        len(units),
                        max(0, (len(units) * (cum[k + 1] - U0)) // (U1 - U0)),
                    )
                    if gi == 4:
                        # next slot is the chunk's first 4-tile S-group; its
                        # PSUM slot frees two small exps back with no PE
                        # lookahead surplus -- keep the queue clear so the
                        # S-matmuls land before the ACT drains exp(c,4)
                        continue
                    while emitted < target and unit_ready(k):
                        emit_unit()
                while emitted < len(units):
                    emit_unit()

            if reps == 1:
                body()
            else:
                with tc.For_i(0, reps, 1):
                    body()

    nc.compile()
    return nc


def _prep_weights(Wkv, bkv, Wp, bp):
    Wkv = np.asarray(Wkv, np.float32)
    bkv = np.asarray(bkv, np.float32)
    Wp = np.asarray(Wp, np.float32)
    bp = np.asarray(bp, np.float32)
    wkb = np.zeros((D, 130), np.float16)
    wkb[:, :D] = Wkv[:, :D].astype(np.float16)
    wkb[:, D] = bkv[:D].astype(np.float16)
    wvp = np.ascontiguousarray((Wkv[:, D:] @ Wp).astype(np.float16))
    bpe_row = bkv[D:] @ Wp + bp
    bpe = np.ascontiguousarray(np.tile(bpe_row[None, :], (D, 1)))
    return wkb, wvp, bpe


_NC_CACHE = {}


def kernel(x, q_global, Wkv, bkv, Wp, bp):
    xt = np.asarray(x, np.float32).astype(np.float16).transpose(0, 2, 1)
    qt = np.asarray(q_global, np.float32).astype(np.float16).transpose(0, 2, 1)
    wkb, wvp, bpe = _prep_weights(Wkv, bkv, Wp, bp)

    if 1 not in _NC_CACHE:
        _NC_CACHE[1] = build(reps=1)
    nc = _NC_CACHE[1]

    in_maps = [
        {
            "xt": np.ascontiguousarray(xt[b]),
            "qt": np.ascontiguousarray(qt[b]),
            "wkb": wkb,
            "wvp": wvp,
            "bpe": bpe,
        }
        for b in range(B)
    ]
    res = run_bass_kernel_spmd(nc, in_maps, core_ids=list(range(B)))
    return np.stack([res.results[b]["out"] for b in range(B)], axis=0)


# revision 36
# speedup vs baseline: 1.0274x; 1.0238x over previous
"""Trainium2 Bass kernel for nn_AttnGlobal (B=8, N=4096, DIM=128).

reference:
    kv = x @ Wkv + bkv ; k, v = split(kv)
    q = q_global / sqrt(d)
    scores = einsum("bnd,bmd->bnm", k, q)       # softmax over m
    attn = softmax(scores, axis=-1)
    out = einsum("bnm,bmd->bnd", attn, v) @ Wp + bp

Sharding: pure data-parallel over B across the 8 cores (one batch each).

Host-side algebra folds:
    w   = x @ (Wv @ Wp)            (since attn @ (x@Wv) @ Wp = attn @ (x@(Wv@Wp)))
    bpe = bv @ Wp + bp             (since rows of attn sum to 1)

Per-core dataflow:
    xT, qT  : host-pretransposed fp16 inputs        [d, n] / [d, m]
    kT      = Wk.T @ xT + bk                        [d, n]   fp16
    S.T     = qT.T-tiles @ kT                       [m, n] tiles in PSUM (fp32)
    E.T     = exp(S.T / sqrt(d))                    fp16, ACT straight from PSUM
    U_aug   = E @ [w | 1]                           [n, 129] accumulated in PSUM
    out     = U[:, :128] * (1 / U[:, 128]) + bpe    DVE, then DMA out

Schedule: the ACT engine's exp stream (16.8M elems/core @ 1 elem/cyc/lane)
is the hard floor (~128us). Everything else is arranged to keep ACT
gapless:

- S-group matmuls are slot-paced: S(c,i) can only run once exp(c,i-2) has
  freed its PSUM slot (two pools in strict alternation, even group count
  per chunk so the alternation is seamless across chunk boundaries).
  U-work is emitted as small 2-tile units threaded between S-groups by an
  ACT-time-proportional owed-work scheduler, deferred past the scratch
  phase so the shared "ps" pool ring (scratch first, then long-held U
  accumulators) can never deadlock the in-order PE queue.
- Input DMAs are cut into pieces sized to land just before their
  need-time, spread over the sync/gpsimd/scalar queues (~30-45GB/s each):
  qT alone outruns one queue during window 0 so its pieces alternate
  between gpsimd and scalar; the kT bias rides inside the wkb transfer
  (a separate 128x4B-descriptor DMA took ~3us and stalled the first exp).
- kT and w-tile scratch matmuls execute inside window 0-1's DMA-bound
  slack; per-chunk outputs leave via one 3D DMA from an SBUF staging
  buffer.
"""

import os
import sys

try:
    import concourse  # noqa: F401  (resolvable via PYTHONPATH on axon images)
except ImportError:
    for _p in ("/opt/trn_rl_repo", os.path.expanduser("~/.axon_site/_ro/trn_rl_repo")):
        if os.path.isdir(_p) and _p not in sys.path:
            sys.path.append(_p)

import numpy as np

import concourse.bacc as bacc
import concourse.mybir as mybir
from concourse.bass_utils import run_bass_kernel_spmd
from concourse.tile import TileContext

B, N, D = 8, 4096, 128
NT = N // 128          # 32 row tiles
NC = N // 512          # 8 column chunks
F32 = mybir.dt.float32
F16 = mybir.dt.float16
EXP_SCALE = 1.0 / float(np.sqrt(D))

# 12 score-groups per chunk; even index -> st2 pool (max 2 tiles), odd ->
# st4 (max 4 tiles). Small groups lead each chunk so chunk 0's early exps
# only need the first qT DMA pieces; the even count keeps the st2/st4
# alternation seamless across chunk boundaries.
GROUPS = [2, 2, 2, 2, 2, 4, 2, 4, 2, 4, 2, 4]
assert sum(GROUPS) == NT
NG = len(GROUPS)
GSTART = [sum(GROUPS[:i]) for i in range(NG)]


def _chunk_groups(c):
    return list(zip(GSTART, GROUPS))


def build(reps: int = 1):
    """Build and compile the per-core Bass program (identical on all cores)."""
    nc = bacc.Bacc("TRN2", target_bir_lowering=False)

    xt = nc.dram_tensor("xt", [D, N], F16, kind="ExternalInput")
    qt = nc.dram_tensor("qt", [D, N], F16, kind="ExternalInput")
    wkb = nc.dram_tensor("wkb", [D, 130], F16, kind="ExternalInput")
    wvp = nc.dram_tensor("wvp", [D, D], F16, kind="ExternalInput")
    bpe = nc.dram_tensor("bpe", [D, D], F32, kind="ExternalInput")  # row-tiled bias
    out = nc.dram_tensor("out", [N, D], F32, kind="ExternalOutput")

    with TileContext(nc) as tc:
        xTc = [nc.alloc_sbuf_tensor(f"xT{c}", [128, 512], F16) for c in range(NC)]
        qTp = [nc.alloc_sbuf_tensor(f"qT{p}", [128, 512], F16) for p in range(NC)]
        kTc = [nc.alloc_sbuf_tensor(f"kT{c}", [128, 512], F16) for c in range(NC)]
        w_aug = nc.alloc_sbuf_tensor("w_aug", [128, NT, 130], F16)
        ET = [nc.alloc_sbuf_tensor(f"et{i}", [128, NT, 512], F16) for i in range(3)]
        warm_sb = nc.alloc_sbuf_tensor("warm_sb", [128, 512], F16)
        wkb_sb = nc.alloc_sbuf_tensor("wkb_sb", [128, 130], F16)
        wvp_sb = nc.alloc_sbuf_tensor("wvp_sb", [128, 128], F16)
        stage = [nc.alloc_sbuf_tensor(f"stg{i}", [128, 4, 128], F32)
                 for i in range(2)]
        bk32 = nc.alloc_sbuf_tensor("bk32", [128, 1], F32)
        bpe_sb = nc.alloc_sbuf_tensor("bpe_sb", [128, 128], F32)

        # DMA piece plan (each queue streams sequentially at ~45GB/s;
        # ~2.8us per 128KB piece): qT pieces alternate between gpsimd and
        # scalar so the m-tile arrival rate keeps up with the exp stream;
        # xT rides sync (chunk 0 first for kT(0)), with the last two
        # chunks on gpsimd after its qT pieces are out.
        def xp(c):
            return (xt[:, c * 512:(c + 1) * 512], xTc[c][:])

        def qp(p):
            return (qt[:, p * 512:(p + 1) * 512], qTp[p][:])

        nc.sync.dma_start(xTc[0][:, :256], xt[:, 0:256])
        nc.sync.dma_start(xTc[0][:, 256:], xt[:, 256:512])
        for s_, d_ in [xp(1), xp(2), xp(3), xp(4), xp(5)]:
            nc.sync.dma_start(d_, s_)
        nc.gpsimd.dma_start(qTp[0][:, :256], qt[:, 0:256])
        nc.gpsimd.dma_start(qTp[0][:, 256:], qt[:, 256:512])
        nc.gpsimd.dma_start(wvp_sb[:], wvp[:])
        for s_, d_ in [qp(2), qp(4), qp(6), xp(6), xp(7)]:
            nc.gpsimd.dma_start(d_, s_)
        nc.scalar.dma_start(wkb_sb[:], wkb[:])
        for s_, d_ in [qp(1), qp(3), qp(5), qp(7)]:
            nc.scalar.dma_start(d_, s_)
        nc.scalar.dma_start(bpe_sb[:], bpe[:])

        with (
            tc.tile_pool(name="outp", bufs=4) as outp,
            tc.tile_pool(name="small", bufs=4) as small,
            tc.tile_pool(name="ps", bufs=2, space="PSUM") as psh,
            tc.tile_pool(name="st4", bufs=1, space="PSUM") as st4,
            tc.tile_pool(name="st2", bufs=1, space="PSUM") as st2,
        ):
            uacc = {}

            def s_group(c, mt, g, gi):
                """scores S.T [m-tiles mt..mt+g, n-chunk c] -> exp -> E.T"""
                pool, tag = (st2, "st2") if gi % 2 == 0 else (st4, "st4")
                stp = pool.tile([128, g * 512], F32, tag=tag)
                for i in range(g):
                    m = mt + i
                    nc.tensor.matmul(
                        stp[:, i * 512:(i + 1) * 512],
                        qTp[m // 4][:, (m % 4) * 128:(m % 4 + 1) * 128],
                        kTc[c][:],
                    )
                nc.scalar.activation(
                    ET[c % 3][:, mt:mt + g, :],
                    stp[:],
                    mybir.ActivationFunctionType.Exp,
                    scale=EXP_SCALE,
                )

            def u_unit(c, j):
                """U += E.T-tiles[2j..2j+1].T @ [w | 1] for output chunk c."""
                if c not in uacc:
                    upa = psh.tile([128, 512], F32, tag="ps")
                    upb = psh.tile([128, 512], F32, tag="ps")
                    uacc[c] = (upa, upb)
                ups = uacc[c]
                buf = ET[c % 3]
                for t in (2 * j, 2 * j + 1):
                    for jj in range(4):
                        up = ups[jj // 2]
                        off = 129 * (jj % 2)
                        nc.tensor.matmul(
                            up[:, off:off + 129],
                            buf[:, t, jj * 128:(jj + 1) * 128],
                            w_aug[:, t, :129],
                            start=(t == 0 and jj % 2 == 0),
                            stop=(t == NT - 1 and jj % 2 == 1),
                        )

            def u_final(c):
                """normalize U by its ones-column, add bias, DMA out."""
                ups = uacc.pop(c)
                stg = stage[c % 2]
                last = c == NC - 1
                for j in range(4):
                    up = ups[j // 2]
                    off = 129 * (j % 2)
                    rec = small.tile([128, 1], F32, tag="rec")
                    nc.vector.reciprocal(rec[:], up[:, off + 128:off + 129])
                    nc.vector.scalar_tensor_tensor(
                        stg[:, j, :],
                        up[:, off:off + 128],
                        rec[:],
                        bpe_sb[:],
                        mybir.AluOpType.mult,
                        mybir.AluOpType.add,
                    )
                    if last:
                        # tail chunk: ship each row-tile as soon as its
                        # normalize lands instead of waiting for all four
                        row = c * 512 + j * 128
                        nc.sync.dma_start(out[row:row + 128, :], stg[:, j, :])
                if not last:
                    nc.sync.dma_start(
                        out[c * 512:(c + 1) * 512, :].rearrange(
                            "(j p) d -> p j d", p=128
                        ),
                        stg[:],
                    )

            def kT_ps(c):
                kt = psh.tile([128, 512], F32, tag="ps")
                nc.tensor.matmul(kt[:], wkb_sb[:, :128], xTc[c][:])
                nc.vector.tensor_scalar_add(kTc[c][:], kt[:], bk32[:])

            def w_quad(q):
                # w-tiles 4q..4q+3; one strided DVE copy into w_aug
                wp = psh.tile([128, 512], F32, tag="ps")
                for i in range(4):
                    t = 4 * q + i
                    nc.tensor.matmul(
                        wp[:, i * 128:(i + 1) * 128],
                        xTc[q][:, i * 128:(i + 1) * 128],
                        wvp_sb[:],
                    )
                nc.vector.tensor_copy(w_aug[:, 4 * q:4 * q + 4, :128], wp[:])

            def body(_iv=None):
                # HAM warmup: data-independent matmuls keep the PE busy
                # while the first input DMAs land, so the clock gate
                # un-throttles (K=4/8 -> 8/8) before the real work starts.
                nc.vector.memset(warm_sb[:], 0.0)
                warm = psh.tile([128, 512], F32, tag="ps")
                for _ in range(6):
                    nc.tensor.matmul(warm[:], warm_sb[:, :128], warm_sb[:])
                nc.vector.memset(w_aug[:, :, 128:129], 1.0)
                nc.vector.tensor_copy(bk32[:], wkb_sb[:, 128:129])
                kt0 = psh.tile([128, 512], F32, tag="ps")
                nc.tensor.matmul(kt0[:, :256], wkb_sb[:, :128], xTc[0][:, :256])
                nc.tensor.matmul(kt0[:, 256:], wkb_sb[:, :128], xTc[0][:, 256:])
                nc.vector.tensor_scalar_add(kTc[0][:], kt0[:], bk32[:])

                # kT/w-quad scratch is threaded through the early slot
                # stream (window 0-1 exp pace is DMA-arrival-bound, so the
                # scratch matmuls execute inside its slack), each placed at
                # a slot by which its xT piece has landed. U-units are
                # deferred until after the last scratch emission, which
                # guarantees every "ps"-ring scratch allocation precedes
                # the first long-held U accumulator: the in-order PE queue
                # can never deadlock on the pool ring.
                wq_done = [0]

                def WQ(q):
                    def f():
                        w_quad(q)
                        wq_done[0] = q + 1
                    return f

                scratch = {
                    1: [WQ(0)],
                    2: [lambda: kT_ps(1)],
                    3: [WQ(1)],
                    4: [lambda: kT_ps(2)],
                    5: [WQ(2)],
                    6: [lambda: kT_ps(3)],
                    7: [WQ(3)],
                    12: [WQ(4)],
                    13: [WQ(5)],
                    14: [WQ(6), lambda: kT_ps(4)],
                    15: [WQ(7), lambda: kT_ps(6)],
                    16: [lambda: kT_ps(5)],
                    17: [lambda: kT_ps(7)],
                }

                seq = [(c, gi) for c in range(NC) for gi in range(NG)]
                tile_group = {}   # (c, mt) -> global slot index of its group
                for k, (c, gi) in enumerate(seq):
                    mt, g = _chunk_groups(c)[gi]
                    for t in range(mt, mt + g):
                        tile_group[(c, t)] = k

                # ACT-time-proportional unit pacing, starting after the
                # last scratch slot (17): weight per slot ~ FD + overhead
                wts = [GROUPS[gi] * 512 + 300 for (c, gi) in seq]
                cum = [0]
                for w_ in wts:
                    cum.append(cum[-1] + w_)
                U0, U1 = cum[18], cum[len(seq)]

                units = [(c, j) for c in range(NC) for j in range(NT // 2)]
                emitted = 0

                def unit_ready(k):
                    if emitted >= len(units):
                        return False
                    c, j = units[emitted]
                    if tile_group[(c, 2 * j + 1)] > k - 2:
                        return False
                    return (2 * j + 1) // 4 < wq_done[0] or c > 0

                def emit_unit():
                    nonlocal emitted
                    uc, uj = units[emitted]
                    u_unit(uc, uj)
                    emitted += 1
                    if uj == NT // 2 - 1:
                        u_final(uc)

                for k, (c, gi) in enumerate(seq):
                    mt, g = _chunk_groups(c)[gi]
                    s_group(c, mt, g, gi)
                    for fn in scratch.get(k, ()):
                        fn()
                    target = min(
                        len(units),
                        max(0, (len(units) * (cum[k + 1] - U0)) // (U1 - U0)),
                    )
                    if gi == 4:
                        # next slot is the chunk's first 4-tile S-group; its
                        # PSUM slot frees two small exps back with no PE
                        # lookahead surplus -- keep the queue clear so the
                        # S-matmuls land before the ACT drains exp(c,4)
                        continue
                    while emitted < target and unit_ready(k):
                        emit_unit()
                while emitted < len(units):
                    emit_unit()

            if reps == 1:
                body()
            else:
                with tc.For_i(0, reps, 1):
                    body()

    nc.compile()
    return nc


def _prep_weights(Wkv, bkv, Wp, bp):
    Wkv = np.asarray(Wkv, np.float32)
    bkv = np.asarray(bkv, np.float32)
    Wp = np.asarray(Wp, np.float32)
    bp = np.asarray(bp, np.float32)
    wkb = np.zeros((D, 130), np.float16)
    wkb[:, :D] = Wkv[:, :D].astype(np.float16)
    wkb[:, D] = bkv[:D].astype(np.float16)
    wvp = np.ascontiguousarray((Wkv[:, D:] @ Wp).astype(np.float16))
    bpe_row = bkv[D:] @ Wp + bp
    bpe = np.ascontiguousarray(np.tile(bpe_row[None, :], (D, 1)))
    return wkb, wvp, bpe


_NC_CACHE = {}


def kernel(x, q_global, Wkv, bkv, Wp, bp):
    xt = np.asarray(x, np.float32).astype(np.float16).transpose(0, 2, 1)
    qt = np.asarray(q_global, np.float32).astype(np.float16).transpose(0, 2, 1)
    wkb, wvp, bpe = _prep_weights(Wkv, bkv, Wp, bp)

    if 1 not in _NC_CACHE:
        _NC_CACHE[1] = build(reps=1)
    nc = _NC_CACHE[1]

    in_maps = [
        {
            "xt": np.ascontiguousarray(xt[b]),
            "qt": np.ascontiguousarray(qt[b]),
            "wkb": wkb,
            "wvp": wvp,
            "bpe": bpe,
        }
        for b in range(B)
    ]
    res = run_bass_kernel_spmd(nc, in_maps, core_ids=list(range(B)))
    return np.stack([res.results[b]["out"] for b in range(B)], axis=0)
